# revision 20
# baseline (speedup 1.0000x reference)
"""Biaffine labeler kernel for 8 Trainium2 NeuronCores.

Computation (full shapes):
    dep  [2, 2048, 1024], head [2, 2049, 1024], head_indices [2, 2048]
    dep_label  = dep @ dep_W.T + dep_b                    [2, 2048, 512]
    selected   = (head gathered at head_indices) @ head_W.T + head_b
    logits[b,t,n] = dep_label[b,t,:] @ W[n] @ selected[b,t,:] + bias[n]

Sharding: data-parallel over (b, t): core c handles b = c // 4 and the
512-token range starting at (c % 4) * 512.  W / projections replicated.

Host prep: the head-row gather runs on the host (head_indices is known),
the label bias is added on the host after the gather, and all matmul
inputs are pre-cast to bf16 and pre-tiled into device layout, including
W (26 MB bf16 instead of 52 MB fp32 streamed through SWDGE).

Per-core device program:
    1. HWDGE input DMAs split across the two rings
       (sync: dep, sel; act: proj weights + biases)
    2. projections on PE with biases folded in as K=1 rank-1 matmuls:
       dep_labelT [512e, 512t] and selected [512t, 512e]
    3. biaffine: labels in groups of 4; for each (group, token-chunk),
       the j-loop streams W[n] chunks with the SAME stationary
       dep_labelT chunk for 4 consecutive matmuls (LDWEIGHTS reuse —
       walrus --enable-ldw-opt is force-enabled via run_command patch),
       accumulating A_n = dep_label @ W[n] into 4 PSUM banks
       (8-bank ping-pong across token chunks)
    4. DVE scalar_tensor_tensor + free-dim accumulator computes
       logits[t,n] = sum_e A_n[t,e]*sel[t,e] per (label, token chunk)
    5. W[n] tiles stream via HWDGE on both rings (sync/act alternating),
       12-label SBUF ring, 3-group prefetch lookahead
"""

import sys

for _p in ("/opt/trn_rl_repo", "/root/.axon_site/_ro/trn_rl_repo"):
    if _p not in sys.path:
        sys.path.append(_p)

from contextlib import ExitStack

import ml_dtypes
import numpy as np

BF16NP = ml_dtypes.bfloat16

import concourse.bass as bass  # noqa: F401
import concourse.bass_utils as bass_utils
import concourse.mybir as mybir
import concourse.tile as tile
from concourse import bacc
from concourse.bass_utils import run_bass_kernel_spmd

import bass_rust as _bass_rust

B, T, D = 2, 2048, 1024
E = 512            # label-space dim (D // 2)
NLAB = 50
NCORES = 8
TLOC = (B * T) // NCORES   # 512 tokens per core
TP = TLOC // 128           # 4 token chunks
DP = D // 128              # 8 contraction chunks for the projections
EP = E // 128              # 4 chunks of the label dim

F32 = mybir.dt.float32
BF16 = mybir.dt.bfloat16

GROUP = 4                  # labels per PSUM group
WBUFS = 16                 # W tiles resident (4 groups)
LOOKAHEAD = 4              # W prefetch distance in groups


def _dedupe_ldweights(nc):
    """Remove LDWEIGHTS whose stationary AP equals the immediately
    preceding one.  With the label-inner matmul ordering, 4 consecutive
    matmuls share the stationary operand; bass emits one LDWEIGHTS per
    matmul unconditionally, and each costs ~46ns of PE issue time.
    Safe here because no SBUF region used as a stationary operand is
    ever rewritten.  Deps of a dropped LDWEIGHTS move to the next
    instruction (its matmul)."""
    for f in nc.m.functions:
        for blk in f.blocks:
            insts = blk.instructions
            last_sig = None
            newlist = []
            pending = None
            changed = False
            for inst in insts:
                if isinstance(inst, _bass_rust.InstLdweights):
                    sig = str(inst.ins[0]).split("bass_ap=")[0]
                    if sig == last_sig:
                        pending = inst
                        changed = True
                        continue
                    last_sig = sig
                if pending is not None:
                    inst.merge_dependencies_from(pending)
                    pending = None
                newlist.append(inst)
            if changed:
                del insts[:]
                insts.extend(newlist)


def build_program():
    nc = bacc.Bacc("TRN2", target_bir_lowering=False, debug=False,
                   num_devices=NCORES)

    dep_T = nc.dram_tensor("dep_T", [128, DP, TLOC], BF16,
                           kind="ExternalInput").ap()
    selT = nc.dram_tensor("selT", [128, DP, TLOC], BF16,
                          kind="ExternalInput").ap()
    depW_T = nc.dram_tensor("depW_T", [128, DP, E], BF16,
                            kind="ExternalInput").ap()
    headW_T = nc.dram_tensor("headW_T", [128, DP, E], BF16,
                             kind="ExternalInput").ap()
    pbias = nc.dram_tensor("pbias", [1, 2 * E], F32,
                           kind="ExternalInput").ap()
    biasn = nc.dram_tensor("biasn", [1, NLAB], F32, kind="ExternalInput").ap()
    Wt = nc.dram_tensor("Wt", [NLAB, 128, EP, E], BF16,
                        kind="ExternalInput").ap()
    logits = nc.dram_tensor("logits", [TLOC, NLAB], F32,
                            kind="ExternalOutput").ap()

    with tile.TileContext(nc) as tc, ExitStack() as ctx:
        pp = ctx.enter_context(tc.tile_pool(name="persist", bufs=1))

        def ptile(shape, dtype, name):
            return pp.tile(shape, dtype, tag=name, name=name)

        ones_r = ptile([1, TLOC], BF16, "ones_r")
        pb_stage = ptile([1, 2 * E], F32, "pb_stage")
        pb_sb = ptile([1, 2 * E], BF16, "pb_sb")
        biasn_f32 = ptile([1, NLAB], F32, "biasn_f32")
        biasn_sb = ptile([1, NLAB], BF16, "biasn_sb")
        bias_bc = ptile([128, NLAB], F32, "bias_bc")
        dep_sT = ptile([128, DP, TLOC], BF16, "dep_sT")   # [d, tok]
        sel_rT = ptile([128, DP, TLOC], BF16, "sel_rT")   # [d, tok]
        depWT = ptile([128, DP, E], BF16, "depWT")        # [d, e]
        headWT = ptile([128, DP, E], BF16, "headWT")      # [d, e]
        dep_lT = ptile([128, EP, TLOC], BF16, "dep_lT")   # [e, tok]
        sel_sb = ptile([128, TP, E], BF16, "sel_sb")      # [tok, e]
        logit_sb = ptile([128, TP, NLAB], F32, "logit_sb")
        logit_out = ptile([128, TP, NLAB], F32, "logit_out")

        # ---- input DMAs; dep/depW interleaved in halves so the dep
        # projection can start on partial data ----
        nc.scalar.dma_start(pb_stage[:], pbias)
        nc.scalar.dma_start(biasn_f32[:], biasn)
        HDP = DP // 2
        nc.sync.dma_start(dep_sT[:, 0:HDP, :], dep_T[:, 0:HDP, :])
        nc.scalar.dma_start(depWT[:, 0:HDP, :], depW_T[:, 0:HDP, :])
        nc.sync.dma_start(dep_sT[:, HDP:DP, :], dep_T[:, HDP:DP, :])
        nc.scalar.dma_start(depWT[:, HDP:DP, :], depW_T[:, HDP:DP, :])
        nc.sync.dma_start(sel_rT[:], selT)
        nc.scalar.dma_start(headWT[:], headW_T)
        nc.vector.memset(ones_r[:], 1.0)
        nc.scalar.copy(pb_sb[:], pb_stage[:])
        nc.scalar.copy(biasn_sb[:], biasn_f32[:])
        depb_sb = pb_sb[:, 0:E]
        headb_sb = pb_sb[:, E:2 * E]

        ps_pool = ctx.enter_context(
            tc.tile_pool(name="ps", bufs=8, space="PSUM"))

        # dep projection -> dep_labelT [e, tok]; bias via K=1 matmul;
        # j-outer so matmuls start as soon as the first halves land
        dps = [ps_pool.tile([128, 512], F32, tag="ps", name=f"psd{i}")
               for i in range(EP)]
        for j in range(DP):
            for i in range(EP):
                nc.tensor.matmul(dps[i][:],
                                 depWT[:, j, i * 128:(i + 1) * 128],
                                 dep_sT[:, j, :],
                                 start=(j == 0), stop=False)
        for i in range(EP):
            nc.tensor.matmul(dps[i][:], depb_sb[:, i * 128:(i + 1) * 128],
                             ones_r[:], start=False, stop=True)
            nc.scalar.copy(dep_lT[:, i, :], dps[i][:])

        # head projection of host-gathered rows -> selected [tok, e]
        hps = [ps_pool.tile([128, 512], F32, tag="ps", name=f"psh{i}")
               for i in range(TP)]
        for j in range(DP):
            for i in range(TP):
                nc.tensor.matmul(hps[i][:],
                                 sel_rT[:, j, i * 128:(i + 1) * 128],
                                 headWT[:, j, :],
                                 start=(j == 0), stop=False)
        for i in range(TP):
            nc.tensor.matmul(hps[i][:], ones_r[:, :128], headb_sb[:],
                             start=False, stop=True)
            nc.scalar.copy(sel_sb[:, i, :], hps[i][:])

        # bias[n] broadcast across partitions: ones[128] x biasn
        psb = ps_pool.tile([128, 512], F32, tag="ps", name="psb")
        nc.tensor.matmul(psb[:, :NLAB], ones_r[:, :128], biasn_sb[:],
                         start=True, stop=True)
        nc.scalar.copy(bias_bc[:], psb[:, :NLAB])

        # ---- biaffine main loop ----
        w_pool = ctx.enter_context(tc.tile_pool(name="wn", bufs=WBUFS))
        dve_dead = ctx.enter_context(tc.tile_pool(name="dd", bufs=2))

        groups = [list(range(s, min(s + GROUP, NLAB)))
                  for s in range(0, NLAB, GROUP)]
        wtiles = {}

        def fetch_group(gi):
            if gi >= len(groups):
                return
            for n in groups[gi]:
                wt = w_pool.tile([128, EP, E], BF16, tag="wn", name=f"w{n}")
                eng = nc.sync if n % 2 == 0 else nc.scalar
                eng.dma_start(wt[:], Wt[n])
                wtiles[n] = wt

        for gi in range(LOOKAHEAD):
            fetch_group(gi)

        for gi, grp in enumerate(groups):
            for i in range(TP):
                pss = [ps_pool.tile([128, 512], F32, tag="ps",
                                    name=f"ps_{gi}_{i}_{k}")
                       for k in range(len(grp))]
                for j in range(EP):
                    lhs = dep_lT[:, j, i * 128:(i + 1) * 128]
                    for k in range(len(grp)):
                        nc.tensor.matmul(pss[k][:], lhs,
                                         wtiles[grp[k]][:, j, :],
                                         start=(j == 0), stop=(j == EP - 1))
                for k, n in enumerate(grp):
                    dead = dve_dead.tile([128, E], BF16, tag="dd",
                                         name=f"dd_{gi}_{i}_{k}")
                    nc.vector.scalar_tensor_tensor(
                        out=dead[:], in0=pss[k][:], scalar=1.0,
                        in1=sel_sb[:, i, :],
                        op0=mybir.AluOpType.mult,
                        op1=mybir.AluOpType.mult,
                        accum_out=logit_sb[:, i, n:n + 1])
            fetch_group(gi + LOOKAHEAD)

        for i in range(TP):
            nc.vector.tensor_add(logit_out[:, i, :], logit_sb[:, i, :],
                                 bias_bc[:])
        nc.sync.dma_start(logits.rearrange("(i p) n -> p i n", p=128),
                          logit_out[:])

    _dedupe_ldweights(nc)
    nc.compile()
    return nc


_NC_CACHE = []


def _get_program():
    if not _NC_CACHE:
        _NC_CACHE.append(build_program())
    return _NC_CACHE[0]


def _dev_layout(a):
    # [x, 1024] operand -> transposed bf16 tile layout [128, 8, x]
    at = np.asarray(a, dtype=np.float32).T.astype(BF16NP)
    return np.ascontiguousarray(
        at.reshape(DP, 128, at.shape[1]).transpose(1, 0, 2))


def make_in_maps(dep, head, head_indices, dep_W, dep_b, head_W, head_b, W,
                 bias):
    dep = np.asarray(dep, dtype=np.float32)
    head = np.asarray(head, dtype=np.float32)
    idx = np.asarray(head_indices)
    W = np.asarray(W, dtype=np.float32)
    pb = np.concatenate([np.asarray(dep_b, dtype=np.float32).ravel(),
                         np.asarray(head_b, dtype=np.float32).ravel()])
    shared = {
        "depW_T": _dev_layout(dep_W),
        "headW_T": _dev_layout(head_W),
        "pbias": np.ascontiguousarray(pb.reshape(1, 2 * E)),
        "biasn": np.ascontiguousarray(bias, dtype=np.float32).reshape(1, NLAB),
        "Wt": np.ascontiguousarray(
            W.reshape(NLAB, EP, 128, E).transpose(0, 2, 1, 3).astype(BF16NP)),
    }
    in_maps = []
    cores_per_b = NCORES // B
    for c in range(NCORES):
        b = c // cores_per_b
        t0 = (c % cores_per_b) * TLOC
        rows = head[b][idx[b, t0:t0 + TLOC]]        # host-side gather
        in_maps.append({
            "dep_T": _dev_layout(dep[b, t0:t0 + TLOC]),
            "selT": _dev_layout(rows),
            **shared,
        })
    return in_maps


def run_sharded(inputs, trace=False):
    """Run the SPMD kernel; returns (full_logits, BassKernelResults)."""
    nc = _get_program()
    in_maps = make_in_maps(
        inputs["dep"], inputs["head"], inputs["head_indices"],
        inputs["dep_W"], inputs["dep_b"], inputs["head_W"],
        inputs["head_b"], inputs["W"], inputs["bias"])
    for attempt in range(3):
        try:
            res = run_bass_kernel_spmd(nc, in_maps, list(range(NCORES)),
                                       trace=trace)
            break
        except Exception:  # transient NRT_EXEC device errors
            if attempt == 2:
                raise
            import time
            time.sleep(5)
    out = np.empty((B, T, NLAB), dtype=np.float32)
    cores_per_b = NCORES // B
    for c in range(NCORES):
        b = c // cores_per_b
        t0 = (c % cores_per_b) * TLOC
        out[b, t0:t0 + TLOC] = res.results[c]["logits"]
    return out, res


def kernel(dep, head, head_indices, mask, dep_W, dep_b, head_W, head_b, W,
           bias):
    out, _ = run_sharded({
        "dep": dep, "head": head, "head_indices": head_indices,
        "dep_W": dep_W, "dep_b": dep_b, "head_W": head_W,
        "head_b": head_b, "W": W, "bias": bias,
    })
    return out


# revision 23
# speedup vs baseline: 1.1844x; 1.1844x over previous
"""Biaffine labeler kernel for 8 Trainium2 NeuronCores.

Computation (full shapes):
    dep  [2, 2048, 1024], head [2, 2049, 1024], head_indices [2, 2048]
    dep_label  = dep @ dep_W.T + dep_b                    [2, 2048, 512]
    selected   = (head gathered at head_indices) @ head_W.T + head_b
    logits[b,t,n] = dep_label[b,t,:] @ W[n] @ selected[b,t,:] + bias[n]

Sharding: data-parallel over (b, t): core c handles b = c // 4 and the
512-token range starting at (c % 4) * 512.  W / projections replicated.

Host prep: the head-row gather runs on the host (head_indices is known),
the label bias is added on the host after the gather, and all matmul
inputs are pre-cast to bf16 and pre-tiled into device layout, including
W (26 MB bf16 instead of 52 MB fp32 streamed through SWDGE).

Per-core device program:
    1. HWDGE input DMAs split across the two rings
       (sync: dep, sel; act: proj weights + biases)
    2. projections on PE with biases folded in as K=1 rank-1 matmuls:
       dep_labelT [512e, 512t] and selected [512t, 512e]
    3. biaffine: labels in groups of 4; for each (group, token-chunk),
       the j-loop streams W[n] chunks with the SAME stationary
       dep_labelT chunk for 4 consecutive matmuls (LDWEIGHTS reuse —
       walrus --enable-ldw-opt is force-enabled via run_command patch),
       accumulating A_n = dep_label @ W[n] into 4 PSUM banks
       (8-bank ping-pong across token chunks)
    4. DVE scalar_tensor_tensor + free-dim accumulator computes
       logits[t,n] = sum_e A_n[t,e]*sel[t,e] per (label, token chunk)
    5. W[n] tiles stream via HWDGE on both rings (sync/act alternating),
       12-label SBUF ring, 3-group prefetch lookahead
"""

import sys

for _p in ("/opt/trn_rl_repo", "/root/.axon_site/_ro/trn_rl_repo"):
    if _p not in sys.path:
        sys.path.append(_p)

from contextlib import ExitStack

import ml_dtypes
import numpy as np

BF16NP = ml_dtypes.bfloat16

import concourse.bass as bass  # noqa: F401
import concourse.bass_utils as bass_utils
import concourse.mybir as mybir
import concourse.tile as tile
from concourse import bacc
from concourse.bass_utils import run_bass_kernel_spmd

import bass_rust as _bass_rust

B, T, D = 2, 2048, 1024
E = 512            # label-space dim (D // 2)
NLAB = 50
NCORES = 8
TLOC = (B * T) // NCORES   # 512 tokens per core
TP = TLOC // 128           # 4 token chunks
DP = D // 128              # 8 contraction chunks for the projections
EP = E // 128              # 4 chunks of the label dim

F32 = mybir.dt.float32
BF16 = mybir.dt.bfloat16

GROUP = 4                  # labels per PSUM group
WGBUFS = 4                 # W group tiles resident
LOOKAHEAD = 3              # W prefetch distance in groups


def _dedupe_ldweights(nc):
    """Remove LDWEIGHTS whose stationary AP equals the immediately
    preceding one.  With the label-inner matmul ordering, 4 consecutive
    matmuls share the stationary operand; bass emits one LDWEIGHTS per
    matmul unconditionally, and each costs ~46ns of PE issue time.
    Safe here because no SBUF region used as a stationary operand is
    ever rewritten.  Deps of a dropped LDWEIGHTS move to the next
    instruction (its matmul)."""
    for f in nc.m.functions:
        for blk in f.blocks:
            insts = blk.instructions
            last_sig = None
            newlist = []
            pending = None
            changed = False
            for inst in insts:
                if isinstance(inst, _bass_rust.InstLdweights):
                    sig = str(inst.ins[0]).split("bass_ap=")[0]
                    if sig == last_sig:
                        pending = inst
                        changed = True
                        continue
                    last_sig = sig
                if pending is not None:
                    inst.merge_dependencies_from(pending)
                    pending = None
                newlist.append(inst)
            if changed:
                del insts[:]
                insts.extend(newlist)


def build_program():
    nc = bacc.Bacc("TRN2", target_bir_lowering=False, debug=False,
                   num_devices=NCORES)

    dep_T = nc.dram_tensor("dep_T", [128, DP, TLOC], BF16,
                           kind="ExternalInput").ap()
    selT = nc.dram_tensor("selT", [128, DP, TLOC], BF16,
                          kind="ExternalInput").ap()
    depW_T = nc.dram_tensor("depW_T", [128, DP, E], BF16,
                            kind="ExternalInput").ap()
    headW_T = nc.dram_tensor("headW_T", [128, DP, E], BF16,
                             kind="ExternalInput").ap()
    pbias = nc.dram_tensor("pbias", [1, 2 * E], F32,
                           kind="ExternalInput").ap()
    biasn = nc.dram_tensor("biasn", [1, NLAB], F32, kind="ExternalInput").ap()
    Wt = nc.dram_tensor("Wt", [NLAB, 128, EP, E], BF16,
                        kind="ExternalInput").ap()
    logits = nc.dram_tensor("logits", [TLOC, NLAB], F32,
                            kind="ExternalOutput").ap()

    with tile.TileContext(nc) as tc, ExitStack() as ctx:
        pp = ctx.enter_context(tc.tile_pool(name="persist", bufs=1))

        def ptile(shape, dtype, name):
            return pp.tile(shape, dtype, tag=name, name=name)

        ones_r = ptile([1, TLOC], BF16, "ones_r")
        pb_stage = ptile([1, 2 * E], F32, "pb_stage")
        pb_sb = ptile([1, 2 * E], BF16, "pb_sb")
        biasn_f32 = ptile([1, NLAB], F32, "biasn_f32")
        biasn_sb = ptile([1, NLAB], BF16, "biasn_sb")
        bias_bc = ptile([128, NLAB], F32, "bias_bc")
        dep_sT = ptile([128, DP, TLOC], BF16, "dep_sT")   # [d, tok]
        sel_rT = ptile([128, DP, TLOC], BF16, "sel_rT")   # [d, tok]
        depWT = ptile([128, DP, E], BF16, "depWT")        # [d, e]
        headWT = ptile([128, DP, E], BF16, "headWT")      # [d, e]
        dep_lT = ptile([128, EP, TLOC], BF16, "dep_lT")   # [e, tok]
        sel_sb = ptile([128, TP, E], BF16, "sel_sb")      # [tok, e]
        logit_sb = ptile([128, TP, NLAB], F32, "logit_sb")
        logit_out = ptile([128, TP, NLAB], F32, "logit_out")

        # ---- input DMAs; dep/depW interleaved in quarters on one ring,
        # in exactly the order the dep projection consumes them ----
        nc.scalar.dma_start(pb_stage[:], pbias)
        nc.scalar.dma_start(biasn_f32[:], biasn)
        QD = 2
        for q in range(0, DP, QD):
            nc.sync.dma_start(dep_sT[:, q:q + QD, :], dep_T[:, q:q + QD, :])
            nc.sync.dma_start(depWT[:, q:q + QD, :], depW_T[:, q:q + QD, :])
        nc.scalar.dma_start(sel_rT[:], selT)
        nc.scalar.dma_start(headWT[:], headW_T)
        nc.vector.memset(ones_r[:], 1.0)
        nc.scalar.copy(pb_sb[:], pb_stage[:])
        nc.scalar.copy(biasn_sb[:], biasn_f32[:])
        depb_sb = pb_sb[:, 0:E]
        headb_sb = pb_sb[:, E:2 * E]

        ps_pool = ctx.enter_context(
            tc.tile_pool(name="ps", bufs=8, space="PSUM"))

        # HAM warm-up: ~3.4us of junk matmuls while the input DMAs land,
        # so the projections start at the full 2.4 GHz PE clock
        psw = ps_pool.tile([128, 512], F32, tag="ps", name="psw")
        for w in range(16):
            nc.tensor.matmul(psw[:], ones_r[:, :128], ones_r[:],
                             start=True, stop=True)

        # dep projection -> dep_labelT [e, tok]; bias via K=1 matmul;
        # j-outer so matmuls start as soon as the first halves land
        dps = [ps_pool.tile([128, 512], F32, tag="ps", name=f"psd{i}")
               for i in range(EP)]
        for j in range(DP):
            for i in range(EP):
                nc.tensor.matmul(dps[i][:],
                                 depWT[:, j, i * 128:(i + 1) * 128],
                                 dep_sT[:, j, :],
                                 start=(j == 0), stop=False)
        for i in range(EP):
            nc.tensor.matmul(dps[i][:], depb_sb[:, i * 128:(i + 1) * 128],
                             ones_r[:], start=False, stop=True)
            nc.scalar.copy(dep_lT[:, i, :], dps[i][:])

        # head projection of host-gathered rows -> selected [tok, e]
        hps = [ps_pool.tile([128, 512], F32, tag="ps", name=f"psh{i}")
               for i in range(TP)]
        for j in range(DP):
            for i in range(TP):
                nc.tensor.matmul(hps[i][:],
                                 sel_rT[:, j, i * 128:(i + 1) * 128],
                                 headWT[:, j, :],
                                 start=(j == 0), stop=False)
        for i in range(TP):
            nc.tensor.matmul(hps[i][:], ones_r[:, :128], headb_sb[:],
                             start=False, stop=True)
            nc.scalar.copy(sel_sb[:, i, :], hps[i][:])

        # bias[n] broadcast across partitions: ones[128] x biasn
        psb = ps_pool.tile([128, 512], F32, tag="ps", name="psb")
        nc.tensor.matmul(psb[:, :NLAB], ones_r[:, :128], biasn_sb[:],
                         start=True, stop=True)
        nc.scalar.copy(bias_bc[:], psb[:, :NLAB])

        # ---- biaffine main loop; W streamed one DMA per label group ----
        w_pool = ctx.enter_context(tc.tile_pool(name="wg", bufs=WGBUFS))
        dve_dead = ctx.enter_context(tc.tile_pool(name="dd", bufs=2))

        groups = [list(range(s, min(s + GROUP, NLAB)))
                  for s in range(0, NLAB, GROUP)]
        wgtiles = {}

        def fetch_group(gi):
            if gi >= len(groups):
                return
            glen = len(groups[gi])
            wg = w_pool.tile([128, GROUP, EP, E], BF16, tag="wg",
                             name=f"wg{gi}")
            eng = nc.sync if gi % 2 == 0 else nc.scalar
            src = Wt[groups[gi][0]:groups[gi][0] + glen]
            eng.dma_start(wg[:, 0:glen, :, :],
                          src.rearrange("g p j e -> p g j e"))
            wgtiles[gi] = wg

        for gi in range(LOOKAHEAD):
            fetch_group(gi)

        logits_r = logits.rearrange("(i p) n -> p i n", p=128)
        for gi, grp in enumerate(groups):
            wg = wgtiles[gi]
            for i in range(TP):
                pss = [ps_pool.tile([128, 512], F32, tag="ps",
                                    name=f"ps_{gi}_{i}_{k}")
                       for k in range(len(grp))]
                for j in range(EP):
                    lhs = dep_lT[:, j, i * 128:(i + 1) * 128]
                    for k in range(len(grp)):
                        nc.tensor.matmul(pss[k][:], lhs,
                                         wg[:, k, j, :],
                                         start=(j == 0), stop=(j == EP - 1))
                for k, n in enumerate(grp):
                    dead = dve_dead.tile([128, E], BF16, tag="dd",
                                         name=f"dd_{gi}_{i}_{k}")
                    nc.vector.scalar_tensor_tensor(
                        out=dead[:], in0=pss[k][:], scalar=1.0,
                        in1=sel_sb[:, i, :],
                        op0=mybir.AluOpType.mult,
                        op1=mybir.AluOpType.mult,
                        accum_out=logit_sb[:, i, n:n + 1])
            fetch_group(gi + LOOKAHEAD)

        for i in range(TP):
            nc.vector.tensor_add(logit_out[:, i, :], logit_sb[:, i, :],
                                 bias_bc[:])
            nc.sync.dma_start(logits_r[:, i, :], logit_out[:, i, :])

    _dedupe_ldweights(nc)
    nc.compile()
    return nc


_NC_CACHE = []


def _get_program():
    if not _NC_CACHE:
        _NC_CACHE.append(build_program())
    return _NC_CACHE[0]


def _dev_layout(a):
    # [x, 1024] operand -> transposed bf16 tile layout [128, 8, x]
    at = np.asarray(a, dtype=np.float32).T.astype(BF16NP)
    return np.ascontiguousarray(
        at.reshape(DP, 128, at.shape[1]).transpose(1, 0, 2))


def make_in_maps(dep, head, head_indices, dep_W, dep_b, head_W, head_b, W,
                 bias):
    dep = np.asarray(dep, dtype=np.float32)
    head = np.asarray(head, dtype=np.float32)
    idx = np.asarray(head_indices)
    W = np.asarray(W, dtype=np.float32)
    pb = np.concatenate([np.asarray(dep_b, dtype=np.float32).ravel(),
                         np.asarray(head_b, dtype=np.float32).ravel()])
    shared = {
        "depW_T": _dev_layout(dep_W),
        "headW_T": _dev_layout(head_W),
        "pbias": np.ascontiguousarray(pb.reshape(1, 2 * E)),
        "biasn": np.ascontiguousarray(bias, dtype=np.float32).reshape(1, NLAB),
        "Wt": np.ascontiguousarray(
            W.reshape(NLAB, EP, 128, E).transpose(0, 2, 1, 3).astype(BF16NP)),
    }
    in_maps = []
    cores_per_b = NCORES // B
    for c in range(NCORES):
        b = c // cores_per_b
        t0 = (c % cores_per_b) * TLOC
        rows = head[b][idx[b, t0:t0 + TLOC]]        # host-side gather
        in_maps.append({
            "dep_T": _dev_layout(dep[b, t0:t0 + TLOC]),
            "selT": _dev_layout(rows),
            **shared,
        })
    return in_maps


def run_sharded(inputs, trace=False):
    """Run the SPMD kernel; returns (full_logits, BassKernelResults)."""
    nc = _get_program()
    in_maps = make_in_maps(
        inputs["dep"], inputs["head"], inputs["head_indices"],
        inputs["dep_W"], inputs["dep_b"], inputs["head_W"],
        inputs["head_b"], inputs["W"], inputs["bias"])
    for attempt in range(3):
        try:
            res = run_bass_kernel_spmd(nc, in_maps, list(range(NCORES)),
                                       trace=trace)
            break
        except Exception:  # transient NRT_EXEC device errors
            if attempt == 2:
                raise
            import time
            time.sleep(5)
    out = np.empty((B, T, NLAB), dtype=np.float32)
    cores_per_b = NCORES // B
    for c in range(NCORES):
        b = c // cores_per_b
        t0 = (c % cores_per_b) * TLOC
        out[b, t0:t0 + TLOC] = res.results[c]["logits"]
    return out, res


def kernel(dep, head, head_indices, mask, dep_W, dep_b, head_W, head_b, W,
           bias):
    out, _ = run_sharded({
        "dep": dep, "head": head, "head_indices": head_indices,
        "dep_W": dep_W, "dep_b": dep_b, "head_W": head_W,
        "head_b": head_b, "W": W, "bias": bias,
    })
    return out
